# revision 25
# baseline (speedup 1.0000x reference)
"""Trainium2 Bass kernel for nn_CNNModel_82222853915196.

Model (per utterance x: (64, 512)):
  multiscale patch features (h in {8,16,32,64}) -> feats (8192,)
  out[t, :] = Wfc @ concat([x[:, t], feats]) + bfc

Factorization: feats is broadcast over t, so
  out = x.T @ Wfc1.T + 1 * (Wfc2 @ feats + cconst).T
with all feature-bias terms folded into cconst on the host.

Everything stays on-chip: the masked-stationary-weight patch matmuls are
restructured so each scale's PSUM tile comes out directly in the layout
the Wfc2 contraction consumes, [f(128 partitions), kt_local*4 + u]:
  - p%4 / p%2 column-parity of a patch index is routed to PSUM partition
    BANDS by splitting the j-offset loop per parity (tile_position).
  - j-offsets are processed in PAIRS via a second copy of x shifted by
    one column living on partitions 64..127 (K=64 -> K=128).
  - h=64 uses W64 as the stationary operand so output partitions are the
    o%128 feature index directly.

DMA sequencing: a DGE queue drains descriptors FIFO, so the sync queue
carries the latency-critical sequence [x-shift, wfc2 chunks 0-5, w64ww
half g=0, chunk 6, w64ww g=1, chunk 7, chunks 8-15] - the C-matmul
stream gets full bandwidth while the PE needs it, and the 2MB w64ww
drains in the window where the PE is busy with C kt-groups 0-5. The PE
emission order interleaves h64 into the half-0 chunk chase the same way.

The frames matmul runs transposed (out partitions = o-block) in fp16
with the C row folded in via a 65th ones-partition; output is written as
outT[400, 2048] fp16 and unscrambled on the host.

Sharding: pure data parallel - 32 utterances -> 8 cores x 4; weights
replicated; no cross-core communication (collectives on this runtime
cost 10-40us of latency per op - measured - so Wfc2 sharding loses).
"""

import os
import sys
from contextlib import ExitStack

import numpy as np

for _p in ("/opt/trn_rl_repo", "/root/.axon_site/_ro/trn_rl_repo"):
    if os.path.isdir(_p) and _p not in sys.path:
        sys.path.insert(0, _p)

import concourse.bass as bass
import concourse.tile as tile
from concourse import bacc, mybir
from concourse.bass_utils import run_bass_kernel_spmd

NCORES = 8
NUTT = 4                 # utterances per core
T = 512
F = 64
OUT = 400
W = NUTT * T             # 2048, free width of the x tile
FP32 = mybir.dt.float32
FP16 = mybir.dt.float16
NPF16 = np.float16


# ---------------------------------------------------------------------------
# host-side weight preparation
# ---------------------------------------------------------------------------

def _build_devindex():
    """devindex[kt, f] = reference flat feature index m in [0, 8192)."""
    devindex = np.full((64, 128), -1, dtype=np.int64)
    # h=8: psF8[f=(p%4)*32 + k*4+o, (p//4)*4+u]
    for k in range(8):
        for p in range(64):
            for o in range(4):
                devindex[p // 4, (p % 4) * 32 + k * 4 + o] = (k * 64 + p) * 4 + o
    # h=16: psF16[f=(p%2)*64 + k*16+o, 16 + p//2]
    for k in range(4):
        for p in range(32):
            for o in range(16):
                devindex[16 + p // 2, (p % 2) * 64 + k * 16 + o] = \
                    2048 + (k * 32 + p) * 16 + o
    # h=32: psF32[f=k*64+o, 32 + p]
    for k in range(2):
        for p in range(16):
            for o in range(64):
                devindex[32 + p, k * 64 + o] = 4096 + (k * 16 + p) * 64 + o
    # h=64: psF64[f=o%128, 48 + p*2 + o//128]
    for p in range(8):
        for o in range(256):
            devindex[48 + p * 2 + o // 128, o % 128] = 6144 + p * 256 + o
    assert devindex.min() >= 0
    return devindex


def _masked(Wh, nk, h, no):
    """w[r, j, k*no+o] = Wh[k, o, (r-k)*h+j] for 0 <= r-k < h else 0."""
    w = np.zeros((64, h, nk * no), dtype=np.float32)
    for k in range(nk):
        for i in range(h):
            w[k + i, :, k * no:(k + 1) * no] = Wh[k].reshape(no, h, h)[:, i, :].T
    return w


def _pair(m):
    """[64, nj, q] -> [128, (nj//2)*q]: row block 0 = even j, block 1 = odd."""
    top = np.ascontiguousarray(m[:, 0::2, :]).reshape(64, -1)
    bot = np.ascontiguousarray(m[:, 1::2, :]).reshape(64, -1)
    return np.concatenate([top, bot], axis=0)


def host_prep(W8, b8, W16, b16, W32, b32, W64, b64, Wfc, bfc):
    f32 = np.float32
    W8 = np.asarray(W8, f32); W16 = np.asarray(W16, f32)
    W32 = np.asarray(W32, f32); W64 = np.asarray(W64, f32)
    Wfc = np.asarray(Wfc, f32)
    b8 = np.asarray(b8, f32); b16 = np.asarray(b16, f32)
    b32 = np.asarray(b32, f32); b64 = np.asarray(b64, f32)
    bfc = np.asarray(bfc, f32)

    w8jj = _pair(_masked(W8, 8, 8, 4))          # [128, 4*32]
    w16jj = _pair(_masked(W16, 4, 16, 16))      # [128, 8*64]
    w32jj = _pair(_masked(W32, 2, 32, 64))      # [128, 16*128]
    # paired then g-major: w64ww[i + 64*par, g*4096 + jp*128 + o']
    #   = W64[g*128 + o', i*64 + 2*jp + par]
    w64ww = _pair(W64.reshape(256, 64, 64).transpose(1, 2, 0))  # [128, 32*256]
    w64ww = np.ascontiguousarray(
        w64ww.reshape(128, 32, 2, 128).transpose(0, 2, 1, 3).reshape(128, 8192))

    devindex = _build_devindex()
    Wfc2 = Wfc[:, 64:]
    # wfc2tf[f, quarter, kt, o'] = Wfc2[quarter*100+o', devindex[kt, f]]
    wfc2t = np.ascontiguousarray(
        Wfc2[:, devindex.reshape(-1)].T.reshape(64, 128, OUT))
    wfc2tf = np.ascontiguousarray(
        wfc2t.transpose(1, 0, 2).reshape(128, 64, 4, 100)
        .transpose(0, 2, 1, 3).reshape(128, 64 * OUT))
    wfc1t4 = np.ascontiguousarray(np.tile(Wfc[:, :64].T, (1, NUTT)))  # [64,1600]

    fb = np.zeros(8192, dtype=np.float64)
    fb[0:2048] = np.broadcast_to(b8[:, None, :], (8, 64, 4)).reshape(-1)
    fb[2048:4096] = np.broadcast_to(b16[:, None, :], (4, 32, 16)).reshape(-1)
    fb[4096:6144] = np.broadcast_to(b32[:, None, :], (2, 16, 64)).reshape(-1)
    fb[6144:8192] = np.broadcast_to(b64[None, :], (8, 256)).reshape(-1)
    cconst = (Wfc2.astype(np.float64) @ fb + bfc.astype(np.float64)).astype(f32)

    return {
        "w8jj": w8jj.astype(NPF16), "w16jj": w16jj.astype(NPF16),
        "w32jj": w32jj.astype(NPF16), "w64ww": w64ww.astype(NPF16),
        "wfc2tf": wfc2tf.astype(NPF16),
        "wfc1t4": wfc1t4.astype(NPF16),
        "cconst": np.ascontiguousarray(cconst.reshape(1, OUT)).astype(NPF16),
        "id4": np.eye(4, dtype=NPF16),
    }


# ---------------------------------------------------------------------------
# device program
# ---------------------------------------------------------------------------

def build_program(trace_sim=False):
    nc = bacc.Bacc("TRN2", target_bir_lowering=False, debug=False)

    dram = dict(
        xh=nc.dram_tensor("xh", [F, W], FP16, kind="ExternalInput"),
        w8jj=nc.dram_tensor("w8jj", [128, 128], FP16, kind="ExternalInput"),
        w16jj=nc.dram_tensor("w16jj", [128, 512], FP16, kind="ExternalInput"),
        w32jj=nc.dram_tensor("w32jj", [128, 2048], FP16, kind="ExternalInput"),
        w64ww=nc.dram_tensor("w64ww", [128, 8192], FP16, kind="ExternalInput"),
        wfc2tf=nc.dram_tensor("wfc2tf", [128, 64 * OUT], FP16, kind="ExternalInput"),
        wfc1t4=nc.dram_tensor("wfc1t4", [F, NUTT * OUT], FP16, kind="ExternalInput"),
        cconst=nc.dram_tensor("cconst", [1, OUT], FP16, kind="ExternalInput"),
        id4=nc.dram_tensor("id4", [4, 4], FP16, kind="ExternalInput"),
        outT=nc.dram_tensor("outT", [OUT, W], FP16, kind="ExternalOutput"),
    )

    with tile.TileContext(nc, trace_sim=trace_sim) as tc:
        with ExitStack() as ctx:
            _emit(nc, tc, ctx, dram)

    nc.compile()
    return nc


def _emit(nc, tc, ctx, dram):
    scalar_dma = nc.scalar.dma_start
    gpsimd_dma = nc.gpsimd.dma_start
    sync_dma = nc.sync.dma_start

    const = ctx.enter_context(tc.tile_pool(name="const", bufs=1))
    stg = ctx.enter_context(tc.tile_pool(name="stg", bufs=2))
    wfc2p = ctx.enter_context(tc.tile_pool(name="wfc2p", bufs=8))
    outp = ctx.enter_context(tc.tile_pool(name="outp", bufs=2))
    ps = ctx.enter_context(tc.tile_pool(name="ps", bufs=2, space="PSUM"))
    psc = ctx.enter_context(tc.tile_pool(name="psc", bufs=1, space="PSUM"))
    psf = ctx.enter_context(tc.tile_pool(name="psf", bufs=4, space="PSUM"))
    psct = ctx.enter_context(tc.tile_pool(name="psct", bufs=1, space="PSUM"))

    # ---- input loads. scalar queue = x + wfc1t4 + out stores; gpsimd
    # (SWDGE) = small weights + C-row bounces; sync queue = the ordered
    # latency-critical stream (x-shift, chunks, w64ww) emitted below.
    # xx: rows 0-63 = x, rows 64-127 = x shifted left one column (j-pairing)
    xx = const.tile([128, W], FP16, tag="xx")
    scalar_dma(xx[0:64, :], dram["xh"].ap())
    sync_dma(xx[64:128, 0:W - 1], dram["xh"].ap()[:, 1:W])

    w8jj = const.tile([128, 128], FP16, tag="w8jj")
    gpsimd_dma(w8jj[:], dram["w8jj"].ap())
    w16jj = const.tile([128, 512], FP16, tag="w16jj")
    gpsimd_dma(w16jj[:], dram["w16jj"].ap())
    w32jj = const.tile([128, 2048], FP16, tag="w32jj")
    scalar_dma(w32jj[:], dram["w32jj"].ap())
    cconst = const.tile([1, OUT], FP16, tag="cconst")
    gpsimd_dma(cconst[:], dram["cconst"].ap())
    ones1 = const.tile([1, NUTT], FP16, tag="ones1")
    nc.vector.memset(ones1[:], 1.0)

    # frames rhs: Wfc1^T tiled per-utt (C is added during the output copies)
    rhs64 = const.tile([64, NUTT * OUT], FP16, tag="rhs64")
    scalar_dma(rhs64[:], dram["wfc1t4"].ap())

    # 4x4 identity for the tiny per-quarter C transposes
    id4 = const.tile([4, 4], FP16, tag="id4")
    gpsimd_dma(id4[:], dram["id4"].ap())
    ctsb = const.tile([100, 4 * NUTT], FP32, tag="ctsb")

    feats = const.tile([128, 64 * NUTT], FP16, tag="feats")
    w64ww = const.tile([128, 8192], FP16, tag="w64ww")

    # ---- sync-queue ordered stream: chunk (quarter, kt-half) pairs with
    # w64ww slotted after q0's pair. Queue FIFO drain gives the C stream
    # priority while the PE chases it, and fills w64ww in the C compute
    # window. Chunk ch = (q, g2): kts g2*32..g2*32+32 for o-quarter q.
    wsrc = dram["wfc2tf"].ap().rearrange("f (c r) -> f c r", c=8)
    chunks = []
    for ch in range(8):
        chunk = wfc2p.tile([128, 32 * 100], FP16, tag="wfc2chunk")
        chunks.append(chunk)
        sync_dma(chunk[:], wsrc[:, ch, :])
        if ch == 1:
            sync_dma(w64ww[:, 0:4096], dram["w64ww"].ap()[:, 0:4096])
            sync_dma(w64ww[:, 4096:8192], dram["w64ww"].ap()[:, 4096:8192])

    # rhs for all masked-scale matmuls: cols (p16, u) at offset j0
    xr = xx[:, :].rearrange("i (u p j) -> i p u j", u=NUTT, p=16, j=32)

    # ---- h=8: psF8[f=(pl*32 + k*4+o), (ph,u)]; bands pl = p%4
    acc = ps.tile([128, 64], FP32, tag="featps")
    for pl in range(4):
        for jp in range(4):
            nc.tensor.matmul(acc[pl * 32:(pl + 1) * 32, :],
                             w8jj[:, jp * 32:(jp + 1) * 32],
                             xr[:, :, :, 8 * pl + 2 * jp],
                             start=(jp == 0), stop=(jp == 3),
                             tile_position=(0, pl * 32))
    nc.vector.tensor_copy(feats[:, 0:64], acc[:])

    # ---- h=16: psF16[f=(pl*64 + k*16+o), (ph,u)]; bands pl = p%2
    acc = ps.tile([128, 64], FP32, tag="featps")
    for pl in range(2):
        for jp in range(8):
            nc.tensor.matmul(acc[pl * 64:(pl + 1) * 64, :],
                             w16jj[:, jp * 64:(jp + 1) * 64],
                             xr[:, :, :, 16 * pl + 2 * jp],
                             start=(jp == 0), stop=(jp == 7),
                             tile_position=(0, pl * 64))
    nc.vector.tensor_copy(feats[:, 64:128], acc[:])

    # ---- h=32: psF32[f=k*64+o, (p,u)]
    acc = ps.tile([128, 64], FP32, tag="featps")
    for jp in range(16):
        nc.tensor.matmul(acc[:],
                         w32jj[:, jp * 128:(jp + 1) * 128],
                         xr[:, :, :, 2 * jp],
                         start=(jp == 0), stop=(jp == 15))
    nc.vector.tensor_copy(feats[:, 128:192], acc[:])

    def h64_feats():
        # ---- h=64: W64 stationary -> psF64[f=o%128, g*32 + (u,p)]
        x64 = xx[:, :].rearrange("i (u p j) -> i u p j", u=NUTT, p=8, j=64)
        acc = ps.tile([128, 64], FP32, tag="featps")
        for g in range(2):
            for jp in range(32):
                nc.tensor.matmul(acc[:, g * 32:(g + 1) * 32],
                                 w64ww[:, g * 4096 + jp * 128: g * 4096 + (jp + 1) * 128],
                                 x64[:, :, :, 2 * jp],
                                 start=(jp == 0), stop=(jp == 31))
        # feats cols for kt=48+p*2+g, u: 192 + p*8 + g*4 + u  <-  acc[(g,u,p)]
        nc.vector.tensor_copy(
            feats[:, 192:256].rearrange("f (p g u) -> f p g u", p=8, g=2, u=NUTT),
            acc[:].rearrange("f (g u p) -> f p g u", g=2, u=NUTT, p=8))

    # ---- C = Wfc2 @ feats, streamed in 8 chunks (4 o-quarters x 2 kt-halves)
    cps = psc.tile([NUTT, OUT], FP32, tag="cps")
    csb = stg.tile([NUTT, OUT], FP16, tag="csb")

    for q in range(4):
        for g2 in range(2):
            if q == 0 and g2 == 1:
                h64_feats()
            chunk = chunks[q * 2 + g2]
            for i in range(32):
                kt = g2 * 32 + i
                nc.tensor.matmul(cps[:, q * 100:(q + 1) * 100],
                                 feats[:, kt * NUTT:(kt + 1) * NUTT],
                                 chunk[:, i * 100:(i + 1) * 100],
                                 start=(kt == 0), stop=False)
        nc.tensor.matmul(cps[:, q * 100:(q + 1) * 100],
                         ones1[:], cconst[:, q * 100:(q + 1) * 100],
                         start=False, stop=True)
        nc.vector.tensor_copy(csb[:, q * 100:(q + 1) * 100],
                              cps[:, q * 100:(q + 1) * 100])

        # ---- frames (transposed): psOT[o-quarter, t] = rhs64-block^T @ x
        # (no C dependency: C is fused into the copies as a per-partition add)
        fsb = outp.tile([100, W], FP16, tag="framesout")
        fpss = []
        for u in range(NUTT):
            fps = psf.tile([100, T], FP32, tag="framesps")
            fpss.append(fps)
            nc.tensor.matmul(
                fps[:],
                rhs64[:, u * OUT + q * 100: u * OUT + (q + 1) * 100],
                xx[0:64, u * T:(u + 1) * T], start=True, stop=True)
        # C^T for this quarter: [100 o-part, 4 u]
        ctp = psct.tile([100, NUTT], FP16, tag="ctp")
        nc.tensor.transpose(ctp[:], csb[0:4, q * 100:(q + 1) * 100], id4[:])
        nc.vector.tensor_copy(ctsb[:, q * 4:(q + 1) * 4], ctp[:])
        for u in range(NUTT):
            cvec = ctsb[:, q * 4 + u: q * 4 + u + 1]
            if u % 2 == 0:
                nc.vector.tensor_scalar_add(fsb[:, u * T:(u + 1) * T],
                                            fpss[u][:], cvec)
            else:
                nc.scalar.activation(fsb[:, u * T:(u + 1) * T], fpss[u][:],
                                     mybir.ActivationFunctionType.Identity,
                                     bias=cvec, scale=1.0)
        scalar_dma(
            bass.AP(tensor=dram["outT"], offset=q * 100 * W,
                    ap=[[W, 100], [1, W]]),
            fsb[:])


_NC_CACHE = None


def _get_nc():
    global _NC_CACHE
    if _NC_CACHE is None:
        _NC_CACHE = build_program()
    return _NC_CACHE


# ---------------------------------------------------------------------------
# entry point
# ---------------------------------------------------------------------------

def run(inputs, trace=False, **kw):
    nc = _get_nc()
    prep = host_prep(inputs["W8"], inputs["b8"], inputs["W16"], inputs["b16"],
                     inputs["W32"], inputs["b32"], inputs["W64"], inputs["b64"],
                     inputs["Wfc"], inputs["bfc"])
    batch = np.asarray(inputs["batch"], np.float32)
    in_maps = []
    for c in range(NCORES):
        xh = np.ascontiguousarray(
            batch[NUTT * c:NUTT * (c + 1)].transpose(1, 0, 2)
            .reshape(F, W).astype(NPF16))
        m = dict(prep)
        m["xh"] = xh
        in_maps.append(m)
    res = run_bass_kernel_spmd(nc, in_maps, core_ids=list(range(NCORES)),
                               trace=trace, **kw)
    outs = []
    for r in res.results:
        o = np.asarray(r["outT"]).astype(np.float32)          # [400, 2048]
        outs.append(o.reshape(OUT, NUTT, T).transpose(1, 2, 0).reshape(-1, OUT))
    return np.concatenate(outs, axis=0), res


def kernel(**inputs):
    out, _ = run(inputs)
    return out
